# revision 1
# baseline (speedup 1.0000x reference)
"""Contrastive (CLIP-style) loss kernel for Trainium2, 8 NeuronCores.

Problem: cxr_feats [8192, 512], ehr_feats [8192, 512], temperature scalar.
  cos_sim = normalize(cxr) @ normalize(ehr).T / temperature        [N, N]
  nll_1 = diag - logsumexp(cos_sim masked-diag, axis=1)
  nll_2 = diag - logsumexp(cos_sim masked-diag, axis=0)
  loss  = -(nll_1 + nll_2).mean()

Sharding: rows of cxr are split across the 8 cores (1024 rows each); every
core holds the full ehr (replicated - the "all-gather one modality" CLIP
strategy, with the gather done host-side for free).  Each core computes its
[1024, 8192] slab of the similarity matrix with fp32r matmuls, takes exp,
row-sums it (fused into the ScalarE activation), and column-sums it with a
ones-vector matmul accumulated in PSUM.  Host combines:
  S1_r = rowsum_r - exp(diag_r);  S2_j = sum_c colsum_c[j] - exp(diag_j)
  loss = -mean(diag - log S1) - mean(diag - log S2)
No max-subtraction needed: |sim| <= ~4 for this data, exp is tame in fp32.
(Entries are cos/temp with cos ~ N(0, 1/512); diag is cos(x_r, y_r), also
small - there is no cancellation in the "subtract the diagonal" trick.)
"""

from contextlib import ExitStack

import numpy as np

import concourse.bass as bass
import concourse.tile as tile
from concourse import bacc
from concourse import mybir
from concourse.bass_utils import run_bass_kernel_spmd
from concourse.masks import make_identity

F32 = mybir.dt.float32
F32R = mybir.dt.float32r
AF = mybir.ActivationFunctionType
ALU = mybir.AluOpType

N = 8192          # rows of each feature matrix
D = 512           # feature dim
NCORES = 8
RPC = N // NCORES  # rows per core (1024)
P = 128            # partitions
NRT = RPC // P     # row tiles per core (8)
NKC = D // P       # contraction chunks (4)
NYT = N // P       # ehr row tiles (64)
CW = 1024          # main-loop column chunk width
NCH = N // CW      # column chunks (8)


def _rsqrt(nc, pool, s_ap, w, name, iters=2):
    """Return an SBUF [128, w] tile holding 1/sqrt(s) (Newton-refined).

    ACT's Rsqrt/Reciprocal LUTs are banned for accuracy; instead use
    vector.reciprocal (iterative divide) + ACT sqrt, then Newton-refine
    r <- r * (1.5 - 0.5 * s * r^2) which only needs mults and one affine.
    """
    inv = pool.tile([P, w], F32, tag=f"{name}_inv")
    nc.vector.reciprocal(inv, s_ap)
    r = pool.tile([P, w], F32, tag=f"{name}_r0")
    nc.scalar.sqrt(r, inv)
    for i in range(iters):
        a = pool.tile([P, w], F32, tag=f"{name}_a{i}")
        nc.vector.tensor_mul(a, r, r)
        b = pool.tile([P, w], F32, tag=f"{name}_b{i}")
        nc.vector.tensor_mul(b, a, s_ap)
        h = pool.tile([P, w], F32, tag=f"{name}_h{i}")
        # h = 1.5 - 0.5 * b   (ACT Copy computes in*scale + bias)
        nc.scalar.activation(h, b, AF.Copy, bias=1.5, scale=-0.5)
        rn = pool.tile([P, w], F32, tag=f"{name}_rn{i}")
        nc.vector.tensor_mul(rn, r, h)
        r = rn
    return r


def _body(ctx, tc, x_d, yx_d, y_d, diag_d, s1_d, cs_d, inv_temp, stage=4):
    nc = tc.nc

    consts = ctx.enter_context(tc.tile_pool(name="consts", bufs=1))
    ident = consts.tile([P, P], F32)
    make_identity(nc, ident)
    ones_f = consts.tile([P, 1], F32)
    nc.vector.memset(ones_f, 1.0)
    ones = consts.tile([P, 1], F32R)
    nc.vector.tensor_copy(ones[:], ones_f[:])

    persist = ctx.enter_context(tc.tile_pool(name="persist", bufs=1))
    Xt = persist.tile([P, NKC * RPC], F32R)   # x^T, chunk k at free [k*RPC + 128*rt]
    Yt = persist.tile([P, NKC * N], F32R)     # (y*t)^T, chunk k at free [k*N + 128*yt]
    sumsq_x = persist.tile([P, NRT], F32)
    sumsq_yx = persist.tile([P, NRT], F32)
    dotxy = persist.tile([P, NRT], F32)
    sumsq_y = persist.tile([P, NYT], F32)
    sx = persist.tile([P, NRT], F32)         # rsqrt(|x|^2) / temp
    diag_sb = persist.tile([P, NRT], F32)
    s1parts = persist.tile([P, NCH * NRT], F32)

    small = ctx.enter_context(tc.tile_pool(name="small", bufs=1))
    stats = ctx.enter_context(tc.tile_pool(name="stats", bufs=5))
    grp = ctx.enter_context(tc.tile_pool(name="grp", bufs=5))
    bounce = ctx.enter_context(tc.tile_pool(name="bounce", bufs=1))
    scr = ctx.enter_context(tc.tile_pool(name="scr", bufs=3))
    epool = ctx.enter_context(tc.tile_pool(name="epool", bufs=2))
    tpsum = ctx.enter_context(tc.tile_pool(name="tpsum", bufs=2, space="PSUM"))
    gpsum = ctx.enter_context(tc.tile_pool(name="gpsum", bufs=2, space="PSUM"))
    cpsum = ctx.enter_context(tc.tile_pool(name="cpsum", bufs=1, space="PSUM"))

    # ---- Phase X stats: sumsq of x rows, paired dot with matching ehr rows
    for rt in range(NRT):
        xt_nat = stats.tile([P, D], F32, tag="snat")
        nc.sync.dma_start(out=xt_nat[:], in_=x_d[rt * P:(rt + 1) * P, :])
        yxt_nat = stats.tile([P, D], F32, tag="snat")
        nc.sync.dma_start(out=yxt_nat[:], in_=yx_d[rt * P:(rt + 1) * P, :])
        sq1 = scr.tile([P, D], F32, tag="scr")
        nc.scalar.activation(sq1, xt_nat[:], AF.Square,
                             accum_out=sumsq_x[:, rt:rt + 1])
        sq2 = scr.tile([P, D], F32, tag="scr")
        nc.scalar.activation(sq2, yxt_nat[:], AF.Square,
                             accum_out=sumsq_yx[:, rt:rt + 1])
        pr = scr.tile([P, D], F32, tag="scr")
        nc.vector.scalar_tensor_tensor(
            out=pr, in0=xt_nat[:], scalar=1.0, in1=yxt_nat[:],
            op0=ALU.mult, op1=ALU.mult, accum_out=dotxy[:, rt:rt + 1])

    # ---- X-side norm finalize: sx = rsqrt(sumsq_x)/temp; diag similarity
    rx = _rsqrt(nc, small, sumsq_x[:], NRT, "rx")
    nc.scalar.mul(sx[:], rx[:], float(inv_temp))
    ryx = _rsqrt(nc, small, sumsq_yx[:], NRT, "ryx")
    dtmp = small.tile([P, NRT], F32, tag="dtmp")
    nc.vector.tensor_mul(dtmp, dotxy[:], sx[:])
    nc.vector.tensor_mul(diag_sb[:], dtmp, ryx[:])
    nc.sync.dma_start(out=diag_d, in_=diag_sb[:])

    if stage < 2:
        return
    # ---- Phase X transpose: groups of 4 row-tiles; one copy per (k, group)
    # so every main-loop matmul operand slice has a single producer.
    for xg in range(NRT // 4):
        g4 = [grp.tile([P, D], F32, tag="gnat", name=f"g4_{i}")
              for i in range(4)]
        for i in range(4):
            rt = xg * 4 + i
            nc.sync.dma_start(out=g4[i][:], in_=x_d[rt * P:(rt + 1) * P, :])
        for k in range(NKC):
            ps = tpsum.tile([P, 512], F32)
            for i in range(4):
                nc.tensor.transpose(ps[:, i * P:(i + 1) * P],
                                    g4[i][:, k * P:(k + 1) * P], ident[:])
            nc.any.tensor_copy(
                out=Xt[:, k * RPC + xg * 512: k * RPC + (xg + 1) * 512],
                in_=ps[:])

    # ---- Phase Y, 4 groups of 16 row-tiles: stats -> rsqrt -> scale+transpose.
    # Grouping (vs one 64-tile batch) lets the transposes and the main loop
    # start as soon as the first group's norms are ready instead of waiting
    # for the whole ehr stats pass.
    for g in range(NYT // 16):
        for yt in range(g * 16, (g + 1) * 16):
            ytile = stats.tile([P, D], F32, tag="snat")
            nc.sync.dma_start(out=ytile[:], in_=y_d[yt * P:(yt + 1) * P, :])
            sc = scr.tile([P, D], F32, tag="scr")
            nc.vector.scalar_tensor_tensor(
                out=sc, in0=ytile[:], scalar=1.0, in1=ytile[:],
                op0=ALU.mult, op1=ALU.mult, accum_out=sumsq_y[:, yt:yt + 1])
        rty = _rsqrt(nc, small, sumsq_y[:, g * 16:(g + 1) * 16], 16, f"rty{g}")
        for yg in range(g * 4, (g + 1) * 4):
            g4 = [grp.tile([P, D], F32, tag="gnat", name=f"g4_{i}")
                  for i in range(4)]
            for i in range(4):
                yt = yg * 4 + i
                nc.sync.dma_start(out=g4[i][:], in_=y_d[yt * P:(yt + 1) * P, :])
                nc.vector.tensor_scalar_mul(g4[i][:], g4[i][:],
                                            rty[:, yt - g * 16:yt - g * 16 + 1])
            for k in range(NKC):
                ps = tpsum.tile([P, 512], F32)
                for i in range(4):
                    nc.tensor.transpose(ps[:, i * P:(i + 1) * P],
                                        g4[i][:, k * P:(k + 1) * P], ident[:])
                nc.any.tensor_copy(
                    out=Yt[:, k * N + yg * 512: k * N + (yg + 1) * 512],
                    in_=ps[:])

    # ---- Main loop: G = x^T-chunks @ y^T, E = exp(G * sx), row/col sums
    for cnk in range(NCH):
        cps = cpsum.tile([1, CW], F32)
        for rt in range(NRT):
            g = gpsum.tile([P, CW], F32)
            for h in range(CW // 512):
                for k in range(NKC):
                    nc.tensor.matmul(
                        g[:, h * 512:(h + 1) * 512],
                        lhsT=Xt[:, k * RPC + rt * P: k * RPC + (rt + 1) * P],
                        rhs=Yt[:, k * N + cnk * CW + h * 512:
                               k * N + cnk * CW + (h + 1) * 512],
                        start=(k == 0), stop=(k == NKC - 1))
            e = epool.tile([P, CW], F32R)
            nc.scalar.activation(
                e, g[:], AF.Exp, scale=sx[:, rt:rt + 1],
                accum_out=s1parts[:, cnk * NRT + rt: cnk * NRT + rt + 1])
            if stage >= 4:
                for h in range(CW // 512):
                    nc.tensor.matmul(
                        cps[:, h * 512:(h + 1) * 512],
                        lhsT=ones[:],
                        rhs=e[:, h * 512:(h + 1) * 512],
                        start=(rt == 0), stop=(rt == NRT - 1))
        if stage >= 4:
            cb = bounce.tile([1, CW], F32, tag="cb")
            nc.any.tensor_copy(out=cb[:], in_=cps[:])
            nc.sync.dma_start(out=cs_d[0:1, cnk * CW:(cnk + 1) * CW], in_=cb[:])

    nc.sync.dma_start(out=s1_d, in_=s1parts[:])


def _build(inv_temp, stage=4):
    nc = bacc.Bacc("TRN2", target_bir_lowering=False, debug=False)
    x_d = nc.dram_tensor("x", [RPC, D], F32, kind="ExternalInput").ap()
    yx_d = nc.dram_tensor("yx", [RPC, D], F32, kind="ExternalInput").ap()
    y_d = nc.dram_tensor("y", [N, D], F32, kind="ExternalInput").ap()
    diag_d = nc.dram_tensor("diag", [P, NRT], F32, kind="ExternalOutput").ap()
    s1_d = nc.dram_tensor("s1parts", [P, NCH * NRT], F32, kind="ExternalOutput").ap()
    cs_d = nc.dram_tensor("colsum", [1, N], F32, kind="ExternalOutput").ap()
    with tile.TileContext(nc) as tc:
        with ExitStack() as ctx:
            _body(ctx, tc, x_d, yx_d, y_d, diag_d, s1_d, cs_d, inv_temp, stage)
    nc.compile()
    return nc


def _combine(results, temp):
    """Host-side reduction of the per-core partials into the scalar loss."""
    diag = np.empty((NCORES, RPC), np.float64)
    rowsum = np.empty((NCORES, RPC), np.float64)
    colsum = np.zeros(N, np.float64)
    for c, r in enumerate(results):
        # [128, NRT] with row = 128*rt + p  ->  transpose to [NRT, 128]
        diag[c] = r["diag"].astype(np.float64).T.reshape(RPC)
        s1 = r["s1parts"].astype(np.float64).reshape(P, NCH, NRT).sum(axis=1)
        rowsum[c] = s1.T.reshape(RPC)
        colsum += r["colsum"].astype(np.float64).reshape(N)
    diag = diag.reshape(N)
    rowsum = rowsum.reshape(N)
    ed = np.exp(diag)
    s1 = rowsum - ed          # row sums exclude the masked diagonal
    s2 = colsum - ed
    nll1 = diag - np.log(s1)
    nll2 = diag - np.log(s2)
    loss = -(nll1.mean() + nll2.mean())
    return np.float32(loss)


def kernel(**inputs):
    x = np.ascontiguousarray(np.asarray(inputs["cxr_feats"], dtype=np.float32))
    y = np.ascontiguousarray(np.asarray(inputs["ehr_feats"], dtype=np.float32))
    temp = float(np.asarray(inputs["temperature"]))
    nc = _build(1.0 / temp)
    in_maps = [
        {"x": x[c * RPC:(c + 1) * RPC], "yx": y[c * RPC:(c + 1) * RPC], "y": y}
        for c in range(NCORES)
    ]
    res = run_bass_kernel_spmd(nc, in_maps, list(range(NCORES)))
    return _combine(res.results, temp)



# revision 7
# speedup vs baseline: 1.3055x; 1.3055x over previous
"""Contrastive (CLIP-style) loss kernel for Trainium2, 8 NeuronCores.

Problem: cxr_feats [8192, 512], ehr_feats [8192, 512], temperature scalar.
  cos_sim = normalize(cxr) @ normalize(ehr).T / temperature        [N, N]
  nll_1 = diag - logsumexp(cos_sim masked-diag, axis=1)
  nll_2 = diag - logsumexp(cos_sim masked-diag, axis=0)
  loss  = -(nll_1 + nll_2).mean()

Sharding: rows of cxr are split across the 8 cores (1024 rows each); every
core holds the full ehr (replicated).  Per core, the [1024, 8192] slab of
the similarity matrix is computed with fp8e4 DoubleRow matmuls (2 fp8
MACs/cell/cycle): both operands are cast to fp8 with static scales (x*16,
y_normalized*64) and the combined 1/(temp*16*64) plus the per-row 1/|x|
ride in the exp's per-partition scale.  exp runs on ACT with the rowsum
accumulated for free; e is written back as fp8 and column-summed with
ones-weight DoubleRow matmuls (two row-tiles per instruction).  The ehr
prep (stats -> rsqrt -> scale-to-fp8 -> PE transpose) is done in 8 groups
of 1024 rows, each group emitted immediately before the main-loop chunk
that consumes it, so PE/ACT/DVE/GpSimd/DMA all overlap and the PE stays
HAM-warm.  Host combines: rowsum/colsum partials minus exp(diag) -> loss.
No max-subtraction needed: |logit| <= ~4 for this data, exp is tame.
"""

from contextlib import ExitStack

import numpy as np

import concourse.bass as bass
import concourse.tile as tile
from concourse import bacc
from concourse import mybir
from concourse.bass_utils import run_bass_kernel_spmd
from concourse.masks import make_identity

F32 = mybir.dt.float32
F8 = mybir.dt.float8e4
BF16 = mybir.dt.bfloat16
AF = mybir.ActivationFunctionType
ALU = mybir.AluOpType
DR = mybir.MatmulPerfMode.DoubleRow

N = 8192           # rows of each feature matrix
D = 512            # feature dim
NCORES = 8
RPC = N // NCORES  # cxr rows per core (1024)
P = 128            # partitions
NRT = RPC // P     # cxr row tiles per core (8)
NKC = D // P       # contraction chunks of 128 (4)
NYT = N // P       # ehr row tiles (64)
GSZ = 8            # ehr tiles per prep group (1024 rows)
NG = NYT // GSZ    # prep groups == main-loop column chunks (8)
CW = 1024          # main-loop column chunk width
SX = 16.0          # fp8 scale for (unnormalized) x
SY = 64.0          # fp8 scale for normalized y


def _rsqrt(nc, pool, s_ap, w, name, iters=2):
    """SBUF [128, w] tile holding 1/sqrt(s), Newton-refined.

    ACT's Rsqrt/Reciprocal LUTs are banned for accuracy; use
    vector.reciprocal + ACT sqrt, then r <- r * (1.5 - 0.5 * s * r^2).
    """
    inv = pool.tile([P, w], F32, tag=f"{name}_inv")
    nc.vector.reciprocal(inv, s_ap)
    r = pool.tile([P, w], F32, tag=f"{name}_r0")
    nc.scalar.sqrt(r, inv)
    for i in range(iters):
        a = pool.tile([P, w], F32, tag=f"{name}_a{i}")
        nc.vector.tensor_mul(a, r, r)
        b = pool.tile([P, w], F32, tag=f"{name}_b{i}")
        nc.vector.tensor_mul(b, a, s_ap)
        h = pool.tile([P, w], F32, tag=f"{name}_h{i}")
        # h = 1.5 - 0.5 * b   (ACT Copy computes in*scale + bias)
        nc.scalar.activation(h, b, AF.Copy, bias=1.5, scale=-0.5)
        rn = pool.tile([P, w], F32, tag=f"{name}_rn{i}")
        nc.vector.tensor_mul(rn, r, h)
        r = rn
    return r


def _body(ctx, tc, x_d, yx_d, y_d, diag_d, s1_d, cs_d, inv_temp):
    nc = tc.nc

    consts = ctx.enter_context(tc.tile_pool(name="consts", bufs=1))
    identb = consts.tile([P, P], BF16)
    make_identity(nc, identb)
    ones8 = consts.tile([P, 2, 16], F8)
    nc.vector.memset(ones8, 1.0)

    persist = ctx.enter_context(tc.tile_pool(name="persist", bufs=1))
    Xt = persist.tile([P, NKC, RPC], F8)     # x^T * SX, chunk k at dim1=k
    Yt = persist.tile([P, NKC, N], F8)       # (y_n*SY)^T
    E = persist.tile([P, NRT, N], F8)        # exp(sim), rt-major
    sumsq_x = persist.tile([P, NRT], F32)
    sumsq_yx = persist.tile([P, NRT], F32)
    dotxy = persist.tile([P, NRT], F32)
    sumsq_y = persist.tile([P, NYT], F32)
    sx = persist.tile([P, NRT], F32)         # rsqrt(|x|^2)/(temp*SX*SY)
    diag_sb = persist.tile([P, NRT], F32)
    s1parts = persist.tile([P, NRT * NG], F32)

    small = ctx.enter_context(tc.tile_pool(name="small", bufs=1))
    xstage = ctx.enter_context(tc.tile_pool(name="xstage", bufs=1))
    ystage = ctx.enter_context(tc.tile_pool(name="ystage", bufs=2))
    y8pool = ctx.enter_context(tc.tile_pool(name="y8pool", bufs=2))
    scr = ctx.enter_context(tc.tile_pool(name="scr", bufs=3))
    bounce = ctx.enter_context(tc.tile_pool(name="bounce", bufs=2))
    tpsum = ctx.enter_context(tc.tile_pool(name="tpsum", bufs=2, space="PSUM"))
    gpsum = ctx.enter_context(tc.tile_pool(name="gpsum", bufs=2, space="PSUM"))
    cpsum = ctx.enter_context(tc.tile_pool(name="cpsum", bufs=2, space="PSUM"))

    # ---- X phase: load, stats, diag, fp8 cast, transpose --------------
    xs = xstage.tile([P, NRT, D], F32)
    nc.sync.dma_start(out=xs[:], in_=x_d.rearrange("(t p) d -> p t d", p=P))
    yxs = xstage.tile([P, NRT, D], F32)
    nc.sync.dma_start(out=yxs[:], in_=yx_d.rearrange("(t p) d -> p t d", p=P))
    for t in range(NRT):
        sq = scr.tile([P, D], F32, tag="scr")
        nc.scalar.activation(sq, xs[:, t, :], AF.Square,
                             accum_out=sumsq_x[:, t:t + 1])
        sq2 = scr.tile([P, D], F32, tag="scr")
        nc.scalar.activation(sq2, yxs[:, t, :], AF.Square,
                             accum_out=sumsq_yx[:, t:t + 1])
        pr = scr.tile([P, D], F32, tag="scr")
        nc.vector.scalar_tensor_tensor(
            out=pr, in0=xs[:, t, :], scalar=1.0, in1=yxs[:, t, :],
            op0=ALU.mult, op1=ALU.mult, accum_out=dotxy[:, t:t + 1])

    rx = _rsqrt(nc, small, sumsq_x[:], NRT, "rx")
    nc.vector.tensor_scalar_mul(sx[:], rx[:], float(inv_temp / (SX * SY)))
    ryx = _rsqrt(nc, small, sumsq_yx[:], NRT, "ryx")
    dtmp = small.tile([P, NRT], F32, tag="dtmp")
    nc.vector.tensor_mul(dtmp, dotxy[:], rx[:])
    dtmp2 = small.tile([P, NRT], F32, tag="dtmp2")
    nc.vector.tensor_scalar_mul(dtmp2, dtmp, float(inv_temp))
    nc.vector.tensor_mul(diag_sb[:], dtmp2, ryx[:])
    nc.sync.dma_start(out=diag_d, in_=diag_sb[:])

    x8 = xstage.tile([P, NRT, D], BF16)
    nc.scalar.activation(x8, xs[:], AF.Copy, scale=SX)
    for k in range(NKC):
        for tq in range(NRT // 4):
            pst = tpsum.tile([P, 512], BF16, tag="tp")
            for i in range(4):
                t = tq * 4 + i
                nc.tensor.transpose(pst[:, i * P:(i + 1) * P],
                                    x8[:, t, k * P:(k + 1) * P], identb[:])
            nc.vector.tensor_copy(out=Xt[:, k, tq * 512:(tq + 1) * 512],
                                  in_=pst[:])

    # ---- Interleaved: per group g, prep ehr rows then GEMM chunk g ----
    for g in range(NG):
        ys = ystage.tile([P, GSZ, D], F32, tag="ys")
        nc.sync.dma_start(
            out=ys[:],
            in_=y_d[g * CW:(g + 1) * CW, :].rearrange("(t p) d -> p t d", p=P))
        for t in range(GSZ):
            sq = scr.tile([P, D], F32, tag="scr")
            nc.vector.scalar_tensor_tensor(
                out=sq, in0=ys[:, t, :], scalar=1.0, in1=ys[:, t, :],
                op0=ALU.mult, op1=ALU.mult,
                accum_out=sumsq_y[:, g * GSZ + t:g * GSZ + t + 1])
        ry = _rsqrt(nc, small, sumsq_y[:, g * GSZ:(g + 1) * GSZ], GSZ,
                    f"ry{g}")
        rys = small.tile([P, GSZ], F32, tag=f"rys{g}")
        nc.vector.tensor_scalar_mul(rys, ry, SY)
        y8 = y8pool.tile([P, GSZ, D], BF16, tag="y8")
        for t in range(GSZ):
            nc.gpsimd.tensor_tensor(out=y8[:, t, :], in0=ys[:, t, :],
                                    in1=rys[:, t:t + 1].broadcast_to((P, D)),
                                    op=mybir.AluOpType.mult)
        for k in range(NKC):
            for tq in range(GSZ // 4):
                pst = tpsum.tile([P, 512], BF16, tag="tp")
                for i in range(4):
                    t = tq * 4 + i
                    nc.tensor.transpose(pst[:, i * P:(i + 1) * P],
                                        y8[:, t, k * P:(k + 1) * P],
                                        identb[:])
                nc.vector.tensor_copy(
                    out=Yt[:, k, g * CW + tq * 512:g * CW + (tq + 1) * 512],
                    in_=pst[:])

        # main-loop chunk g: sim rows x cols [g*CW, (g+1)*CW)
        for rt in range(NRT):
            gp = gpsum.tile([P, CW], F32, tag="g")
            for kp in range(NKC // 2):
                for h in range(CW // 512):
                    nc.tensor.matmul(
                        gp[:, h * 512:(h + 1) * 512],
                        lhsT=Xt[:, 2 * kp:2 * kp + 2, rt * P:(rt + 1) * P],
                        rhs=Yt[:, 2 * kp:2 * kp + 2,
                               g * CW + h * 512:g * CW + (h + 1) * 512],
                        start=(kp == 0), stop=(kp == NKC // 2 - 1),
                        perf_mode=DR)
            nc.scalar.activation(
                E[:, rt, g * CW:(g + 1) * CW], gp[:], AF.Exp,
                scale=sx[:, rt:rt + 1],
                accum_out=s1parts[:, rt * NG + g:rt * NG + g + 1])

    nc.sync.dma_start(out=s1_d, in_=s1parts[:])

    # ---- Colsum end-pass: DoubleRow ones-reduction over rt pairs ------
    for ch in range(NG):
        for h in range(CW // 512):
            cps = cpsum.tile([1, 512], F32, tag="c")
            for pr in range(NRT // 2):
                nc.tensor.matmul(
                    cps[:],
                    lhsT=ones8[:, :, 0:1],
                    rhs=E[:, 2 * pr:2 * pr + 2,
                          ch * CW + h * 512:ch * CW + (h + 1) * 512],
                    start=(pr == 0), stop=(pr == NRT // 2 - 1),
                    perf_mode=DR)
            cb = bounce.tile([1, 512], F32, tag="cb")
            nc.vector.tensor_copy(out=cb[:], in_=cps[:])
            nc.sync.dma_start(
                out=cs_d[0:1, ch * CW + h * 512:ch * CW + (h + 1) * 512],
                in_=cb[:])


def _build(inv_temp):
    nc = bacc.Bacc("TRN2", target_bir_lowering=False, debug=False)
    x_d = nc.dram_tensor("x", [RPC, D], F32, kind="ExternalInput").ap()
    yx_d = nc.dram_tensor("yx", [RPC, D], F32, kind="ExternalInput").ap()
    y_d = nc.dram_tensor("y", [N, D], F32, kind="ExternalInput").ap()
    diag_d = nc.dram_tensor("diag", [P, NRT], F32, kind="ExternalOutput").ap()
    s1_d = nc.dram_tensor("s1parts", [P, NRT * NG], F32,
                          kind="ExternalOutput").ap()
    cs_d = nc.dram_tensor("colsum", [1, N], F32, kind="ExternalOutput").ap()
    with tile.TileContext(nc) as tc:
        with ExitStack() as ctx:
            _body(ctx, tc, x_d, yx_d, y_d, diag_d, s1_d, cs_d, inv_temp)
    nc.compile()
    return nc


def _combine(results):
    """Host-side reduction of the per-core partials into the scalar loss."""
    diag = np.empty((NCORES, RPC), np.float64)
    rowsum = np.empty((NCORES, RPC), np.float64)
    colsum = np.zeros(N, np.float64)
    for c, r in enumerate(results):
        diag[c] = r["diag"].astype(np.float64).T.reshape(RPC)
        s1 = r["s1parts"].astype(np.float64).reshape(P, NRT, NG).sum(axis=2)
        rowsum[c] = s1.T.reshape(RPC)
        colsum += r["colsum"].astype(np.float64).reshape(N)
    diag = diag.reshape(N)
    rowsum = rowsum.reshape(N)
    ed = np.exp(diag)
    s1 = rowsum - ed          # row sums exclude the masked diagonal
    s2 = colsum - ed
    nll1 = diag - np.log(s1)
    nll2 = diag - np.log(s2)
    loss = -(nll1.mean() + nll2.mean())
    return np.float32(loss)


def kernel(**inputs):
    x = np.ascontiguousarray(np.asarray(inputs["cxr_feats"], dtype=np.float32))
    y = np.ascontiguousarray(np.asarray(inputs["ehr_feats"], dtype=np.float32))
    temp = float(np.asarray(inputs["temperature"]))
    nc = _build(1.0 / temp)
    in_maps = [
        {"x": x[c * RPC:(c + 1) * RPC], "yx": y[c * RPC:(c + 1) * RPC], "y": y}
        for c in range(NCORES)
    ]
    res = run_bass_kernel_spmd(nc, in_maps, list(range(NCORES)))
    return _combine(res.results)


# revision 9
# speedup vs baseline: 1.4792x; 1.1331x over previous
"""Contrastive (CLIP-style) loss kernel for Trainium2, 8 NeuronCores.

Problem: cxr_feats [8192, 512], ehr_feats [8192, 512], temperature scalar.
  cos_sim = normalize(cxr) @ normalize(ehr).T / temperature        [N, N]
  nll_1 = diag - logsumexp(cos_sim masked-diag, axis=1)
  nll_2 = diag - logsumexp(cos_sim masked-diag, axis=0)
  loss  = -(nll_1 + nll_2).mean()

Sharding: rows of cxr are split across the 8 cores (1024 rows each); every
core holds the full ehr (replicated).  Per core, the [1024, 8192] slab of
the similarity matrix is computed with fp8e4 DoubleRow matmuls (2 fp8
MACs/cell/cycle): both operands are cast to fp8 with static scales (x*16,
y_normalized*64) and the combined 1/(temp*16*64) plus the per-row 1/|x|
ride in the exp's per-partition scale.  exp runs on ACT with the rowsum
accumulated for free; e is written back as fp8 and column-summed with
ones-weight DoubleRow matmuls (two row-tiles per instruction).  The ehr
prep (stats -> rsqrt -> scale-to-fp8 -> PE transpose) is done in 8 groups
of 1024 rows, each group emitted immediately before the main-loop chunk
that consumes it, so PE/ACT/DVE/GpSimd/DMA all overlap and the PE stays
HAM-warm.  Host combines: rowsum/colsum partials minus exp(diag) -> loss.
No max-subtraction needed: |logit| <= ~4 for this data, exp is tame.
"""

from contextlib import ExitStack

import numpy as np

import concourse.bass as bass
import concourse.tile as tile
from concourse import bacc
from concourse import mybir
from concourse.bass_utils import run_bass_kernel_spmd
from concourse.masks import make_identity

F32 = mybir.dt.float32
F8 = mybir.dt.float8e4
BF16 = mybir.dt.bfloat16
AF = mybir.ActivationFunctionType
ALU = mybir.AluOpType
DR = mybir.MatmulPerfMode.DoubleRow

N = 8192           # rows of each feature matrix
D = 512            # feature dim
NCORES = 8
RPC = N // NCORES  # cxr rows per core (1024)
P = 128            # partitions
NRT = RPC // P     # cxr row tiles per core (8)
NKC = D // P       # contraction chunks of 128 (4)
NYT = N // P       # ehr row tiles (64)
GSZ = 8            # ehr tiles per prep group (1024 rows)
NG = NYT // GSZ    # prep groups == main-loop column chunks (8)
CW = 1024          # main-loop column chunk width
SX = 16.0          # fp8 scale for (unnormalized) x
SY = 64.0          # fp8 scale for normalized y


I32 = mybir.dt.int32


def _rsqrt(nc, pool, s_ap, w, name, iters=3):
    """SBUF [128, w] tile holding 1/sqrt(s), DVE-only.

    Quake fast-inverse-sqrt seed (0x5f3759df bit trick) + Newton
    r <- r * (1.5 - 0.5 * s * r^2).  Avoids ACT's Sqrt LUT entirely so
    the ACT table RAM stays on the exp set (no ~1.3us reload thrash).
    """
    half = pool.tile([P, w], I32, tag=f"{name}_h0")
    nc.vector.tensor_scalar(out=half, in0=s_ap.bitcast(I32), scalar1=1,
                            scalar2=None, op0=ALU.logical_shift_right)
    magic = pool.tile([P, w], I32, tag=f"{name}_mg")
    nc.vector.memset(magic, 0x5F3759DF)
    ri = pool.tile([P, w], I32, tag=f"{name}_ri")
    nc.vector.tensor_tensor(out=ri, in0=magic[:], in1=half[:],
                            op=ALU.subtract)
    r = ri[:].bitcast(F32)
    for i in range(iters):
        a = pool.tile([P, w], F32, tag=f"{name}_a{i}")
        nc.vector.tensor_mul(a, r, r)
        b = pool.tile([P, w], F32, tag=f"{name}_b{i}")
        nc.vector.tensor_mul(b, a, s_ap)
        h = pool.tile([P, w], F32, tag=f"{name}_h{i}")
        nc.vector.tensor_scalar(out=h, in0=b[:], scalar1=-0.5, scalar2=1.5,
                                op0=ALU.mult, op1=ALU.add)
        rn = pool.tile([P, w], F32, tag=f"{name}_rn{i}")
        nc.vector.tensor_mul(rn, r, h)
        r = rn[:]
    return r


def _body(ctx, tc, x_d, yx_d, y_d, diag_d, s1_d, cs_d, inv_temp):
    nc = tc.nc

    consts = ctx.enter_context(tc.tile_pool(name="consts", bufs=1))
    identb = consts.tile([P, P], BF16)
    make_identity(nc, identb)
    ones8 = consts.tile([P, 2, 16], F8)
    nc.vector.memset(ones8, 1.0)

    persist = ctx.enter_context(tc.tile_pool(name="persist", bufs=1))
    Xt = persist.tile([P, NKC, RPC], F8)     # x^T * SX, chunk k at dim1=k
    Yt = persist.tile([P, NKC, N], F8)       # (y_n*SY)^T
    E = persist.tile([P, NRT, N], F8)        # exp(sim), rt-major
    sumsq_x = persist.tile([P, NRT], F32)
    sumsq_yx = persist.tile([P, NRT], F32)
    dotxy = persist.tile([P, NRT], F32)
    sumsq_y = persist.tile([P, NYT], F32)
    sx = persist.tile([P, NRT], F32)         # rsqrt(|x|^2)/(temp*SX*SY)
    diag_sb = persist.tile([P, NRT], F32)
    s1parts = persist.tile([P, NRT * NG], F32)

    small = ctx.enter_context(tc.tile_pool(name="small", bufs=1))
    xstage = ctx.enter_context(tc.tile_pool(name="xstage", bufs=1))
    ystage = ctx.enter_context(tc.tile_pool(name="ystage", bufs=2))
    y8pool = ctx.enter_context(tc.tile_pool(name="y8pool", bufs=2))
    scr = ctx.enter_context(tc.tile_pool(name="scr", bufs=4))
    bounce = ctx.enter_context(tc.tile_pool(name="bounce", bufs=2))
    tpsum = ctx.enter_context(tc.tile_pool(name="tpsum", bufs=2, space="PSUM"))
    gpsum = ctx.enter_context(tc.tile_pool(name="gpsum", bufs=2, space="PSUM"))
    cpsum = ctx.enter_context(tc.tile_pool(name="cpsum", bufs=2, space="PSUM"))

    # ---- X phase: load, stats, diag, fp8 cast, transpose --------------
    xs = xstage.tile([P, NRT, D], F32)
    nc.sync.dma_start(out=xs[:], in_=x_d.rearrange("(t p) d -> p t d", p=P))
    yxs = xstage.tile([P, NRT, D], F32)
    nc.sync.dma_start(out=yxs[:], in_=yx_d.rearrange("(t p) d -> p t d", p=P))
    for t in range(NRT):
        sq = scr.tile([P, D], F32, tag="scr")
        nc.scalar.activation(sq, xs[:, t, :], AF.Square,
                             accum_out=sumsq_x[:, t:t + 1])
        sq2 = scr.tile([P, D], F32, tag="scr")
        nc.scalar.activation(sq2, yxs[:, t, :], AF.Square,
                             accum_out=sumsq_yx[:, t:t + 1])
        pr = scr.tile([P, D], F32, tag="scr")
        nc.vector.scalar_tensor_tensor(
            out=pr, in0=xs[:, t, :], scalar=1.0, in1=yxs[:, t, :],
            op0=ALU.mult, op1=ALU.mult, accum_out=dotxy[:, t:t + 1])

    rx = _rsqrt(nc, small, sumsq_x[:], NRT, "rx")
    nc.vector.tensor_scalar_mul(sx[:], rx[:], float(inv_temp / (SX * SY)))
    ryx = _rsqrt(nc, small, sumsq_yx[:], NRT, "ryx")
    dtmp = small.tile([P, NRT], F32, tag="dtmp")
    nc.vector.tensor_mul(dtmp, dotxy[:], rx[:])
    dtmp2 = small.tile([P, NRT], F32, tag="dtmp2")
    nc.vector.tensor_scalar_mul(dtmp2, dtmp, float(inv_temp))
    nc.vector.tensor_mul(diag_sb[:], dtmp2, ryx[:])
    nc.sync.dma_start(out=diag_d, in_=diag_sb[:])

    x8 = xstage.tile([P, NRT, D], BF16)
    nc.scalar.activation(x8, xs[:], AF.Copy, scale=SX)
    for k in range(NKC):
        for tq in range(NRT // 4):
            pst = tpsum.tile([P, 512], BF16, tag="tp")
            for i in range(4):
                t = tq * 4 + i
                nc.tensor.transpose(pst[:, i * P:(i + 1) * P],
                                    x8[:, t, k * P:(k + 1) * P], identb[:])
            nc.vector.tensor_copy(out=Xt[:, k, tq * 512:(tq + 1) * 512],
                                  in_=pst[:])

    # ---- Interleaved: per group g, prep ehr rows then GEMM chunk g ----
    for g in range(NG):
        ys = ystage.tile([P, GSZ, D], F32, tag="ys")
        nc.sync.dma_start(
            out=ys[:],
            in_=y_d[g * CW:(g + 1) * CW, :].rearrange("(t p) d -> p t d", p=P))
        for t in range(GSZ):
            sq = scr.tile([P, D], F32, tag="scr")
            nc.vector.scalar_tensor_tensor(
                out=sq, in0=ys[:, t, :], scalar=1.0, in1=ys[:, t, :],
                op0=ALU.mult, op1=ALU.mult,
                accum_out=sumsq_y[:, g * GSZ + t:g * GSZ + t + 1])
        ry = _rsqrt(nc, small, sumsq_y[:, g * GSZ:(g + 1) * GSZ], GSZ,
                    f"ry{g}")
        rys = small.tile([P, GSZ], F32, tag=f"rys{g}")
        nc.vector.tensor_scalar_mul(rys, ry, SY)
        y8 = y8pool.tile([P, GSZ, D], BF16, tag="y8")
        for t in range(GSZ):
            nc.gpsimd.tensor_tensor(out=y8[:, t, :], in0=ys[:, t, :],
                                    in1=rys[:, t:t + 1].broadcast_to((P, D)),
                                    op=mybir.AluOpType.mult)
        for k in range(NKC):
            for tq in range(GSZ // 4):
                pst = tpsum.tile([P, 512], BF16, tag="tp")
                for i in range(4):
                    t = tq * 4 + i
                    nc.tensor.transpose(pst[:, i * P:(i + 1) * P],
                                        y8[:, t, k * P:(k + 1) * P],
                                        identb[:])
                nc.vector.tensor_copy(
                    out=Yt[:, k, g * CW + tq * 512:g * CW + (tq + 1) * 512],
                    in_=pst[:])

        # main-loop chunk g: sim rows x cols [g*CW, (g+1)*CW)
        for rt in range(NRT):
            gp = gpsum.tile([P, CW], F32, tag="g")
            for kp in range(NKC // 2):
                for h in range(CW // 512):
                    nc.tensor.matmul(
                        gp[:, h * 512:(h + 1) * 512],
                        lhsT=Xt[:, 2 * kp:2 * kp + 2, rt * P:(rt + 1) * P],
                        rhs=Yt[:, 2 * kp:2 * kp + 2,
                               g * CW + h * 512:g * CW + (h + 1) * 512],
                        start=(kp == 0), stop=(kp == NKC // 2 - 1),
                        perf_mode=DR)
            nc.scalar.activation(
                E[:, rt, g * CW:(g + 1) * CW], gp[:], AF.Exp,
                scale=sx[:, rt:rt + 1],
                accum_out=s1parts[:, rt * NG + g:rt * NG + g + 1])

    nc.sync.dma_start(out=s1_d, in_=s1parts[:])

    # ---- Colsum end-pass: DoubleRow ones-reduction over rt pairs ------
    for ch in range(NG):
        for h in range(CW // 512):
            cps = cpsum.tile([1, 512], F32, tag="c")
            for pr in range(NRT // 2):
                nc.tensor.matmul(
                    cps[:],
                    lhsT=ones8[:, :, 0:1],
                    rhs=E[:, 2 * pr:2 * pr + 2,
                          ch * CW + h * 512:ch * CW + (h + 1) * 512],
                    start=(pr == 0), stop=(pr == NRT // 2 - 1),
                    perf_mode=DR)
            cb = bounce.tile([1, 512], F32, tag="cb")
            nc.vector.tensor_copy(out=cb[:], in_=cps[:])
            nc.sync.dma_start(
                out=cs_d[0:1, ch * CW + h * 512:ch * CW + (h + 1) * 512],
                in_=cb[:])


def _build(inv_temp):
    nc = bacc.Bacc("TRN2", target_bir_lowering=False, debug=False)
    x_d = nc.dram_tensor("x", [RPC, D], F32, kind="ExternalInput").ap()
    yx_d = nc.dram_tensor("yx", [RPC, D], F32, kind="ExternalInput").ap()
    y_d = nc.dram_tensor("y", [N, D], F32, kind="ExternalInput").ap()
    diag_d = nc.dram_tensor("diag", [P, NRT], F32, kind="ExternalOutput").ap()
    s1_d = nc.dram_tensor("s1parts", [P, NRT * NG], F32,
                          kind="ExternalOutput").ap()
    cs_d = nc.dram_tensor("colsum", [1, N], F32, kind="ExternalOutput").ap()
    with tile.TileContext(nc) as tc:
        with ExitStack() as ctx:
            _body(ctx, tc, x_d, yx_d, y_d, diag_d, s1_d, cs_d, inv_temp)
    nc.compile()
    return nc


def _combine(results):
    """Host-side reduction of the per-core partials into the scalar loss."""
    diag = np.empty((NCORES, RPC), np.float64)
    rowsum = np.empty((NCORES, RPC), np.float64)
    colsum = np.zeros(N, np.float64)
    for c, r in enumerate(results):
        diag[c] = r["diag"].astype(np.float64).T.reshape(RPC)
        s1 = r["s1parts"].astype(np.float64).reshape(P, NRT, NG).sum(axis=2)
        rowsum[c] = s1.T.reshape(RPC)
        colsum += r["colsum"].astype(np.float64).reshape(N)
    diag = diag.reshape(N)
    rowsum = rowsum.reshape(N)
    ed = np.exp(diag)
    s1 = rowsum - ed          # row sums exclude the masked diagonal
    s2 = colsum - ed
    nll1 = diag - np.log(s1)
    nll2 = diag - np.log(s2)
    loss = -(nll1.mean() + nll2.mean())
    return np.float32(loss)


def kernel(**inputs):
    x = np.ascontiguousarray(np.asarray(inputs["cxr_feats"], dtype=np.float32))
    y = np.ascontiguousarray(np.asarray(inputs["ehr_feats"], dtype=np.float32))
    temp = float(np.asarray(inputs["temperature"]))
    nc = _build(1.0 / temp)
    in_maps = [
        {"x": x[c * RPC:(c + 1) * RPC], "yx": y[c * RPC:(c + 1) * RPC], "y": y}
        for c in range(NCORES)
    ]
    res = run_bass_kernel_spmd(nc, in_maps, list(range(NCORES)))
    return _combine(res.results)
